# revision 27
# baseline (speedup 1.0000x reference)
"""Mixed causal attention (B=8,L=1024,D=1024,H=16,NS=8) on 8 TRN2 cores.

Sharding: data-parallel over batch (core b owns batch b).  The per-position
(ns) projection weights are sharded by position: core c computes q/k/v for
position 1016+c for ALL batches; an AllToAll routes batch-b rows to core b,
so each core receives exactly its own batch's 8 tail rows.

v4 vs v3 (319us):
  - compacted score layout: each jp-pair's scores land densely packed
    (only causally-valid columns), so exp is ONE activation per (head,
    jp-pair) with no dead columns -- removes ~40% of ACT instruction
    overhead (the +352-cycle fixed cost per ACTIVATE).
  - vb layout is [kpm-replicated 64 | v 64] per head, so the softmax
    denominator comes out of the PV matmul on PSUM partitions 0..63 and
    reciprocal_approx_fast reads PSUM directly: 2-instruction normalize,
    no gpsimd, no staging copy.
  - g0 and g1 attention rounds interleave (g1 r starts once the tails
    land), and the output projection is split by lt (q-position block)
    instead of by et: lt 0-3 touch only the g0 half of oT so they fill
    the ACT-bound g1 rounds; no partial-sum staging buffer at all.
  - V prelude runs dt-major over lt 0-2 so the PE has work while the
    interleaved xT/wv chunks stream in.
"""

import sys
import os
from contextlib import ExitStack

import numpy as np

sys.path.insert(0, "/opt/trn_rl_repo")

import ml_dtypes  # noqa: E402
import concourse.bass as bass  # noqa: E402
import concourse.tile as tile  # noqa: E402
from concourse import bacc, mybir  # noqa: E402
from concourse._compat import with_exitstack  # noqa: E402
from concourse.bass_utils import run_bass_kernel_spmd  # noqa: E402

B, L, D, H, NS = 8, 1024, 1024, 16, 8
HD = D // H          # 64
LS = L - NS          # 1016
NCORES = 8
NEG = -1.0e9
BF = mybir.dt.bfloat16
F32 = mybir.dt.float32
EXP = mybir.ActivationFunctionType.Exp
ADD = mybir.AluOpType.add
MULT = mybir.AluOpType.mult

_CACHE = {}
TRACE = False
DEBUG = False


def _lead(g, j):
    return max(0, j * 128 - g * 512)


def _pcs(g):
    # compacted pt column offset per j block (valid width = 512 - lead)
    offs, o = [], 0
    for j in range(4 if g == 0 else 8):
        offs.append(o)
        o += 512 - _lead(g, j)
    return offs, o


@with_exitstack
def _attention_kernel(ctx: ExitStack, tc: tile.TileContext, aps: dict):
    nc = tc.nc

    sb = ctx.enter_context(tc.tile_pool(name="persist", bufs=1))
    stage = ctx.enter_context(tc.tile_pool(name="stage", bufs=2))
    dram = ctx.enter_context(tc.tile_pool(name="dram", bufs=2, space="DRAM"))
    accp = ctx.enter_context(tc.tile_pool(name="accp", bufs=2, space="PSUM"))

    # ---- persistent SBUF tensors ----
    qT = sb.tile([128, 8 * 1024], BF)      # [e-part, et*1024 + l]
    kT = sb.tile([128, 8 * 1024], BF)
    # [k-part, lt*2048 + h*128 + c]; c 0:64 = kpm replicated (softmax
    # denominator rider on PSUM partitions 0..63), c 64:128 = v-values.
    vb = sb.tile([128, 8 * 2048], BF)
    oT = sb.tile([128, 8 * 1024], BF)      # [e-part, et*1024 + l]
    xtails = sb.tile([128, 64], BF)        # [d-part, dt*8 + bb]
    id8 = sb.tile([8, 8], BF)              # identity (tail transpose matmuls)
    tri = sb.tile([128, 128], F32)         # tri[p,f] = 0 if p<=f else NEG
    kpm = sb.tile([128, 8], F32)           # key-padding 0/1 per [key-in-block, lt]
    ktail8 = sb.tile([8, 1], F32)          # key-padding for tail keys 1016..1023
    nsb = sb.tile([8, 3072], BF)           # my position's q|k|v for all batches
    a2a = sb.tile([8, 3072], BF)           # routed: row n = position n, my batch
    wo = sb.tile([128, 8 * 1024], BF)      # [e-part, et*1024 + e']

    # ---- ACT table warm-up: first ACTIVATE in the scalar stream is an Exp,
    # so the exp_and_others table set loads once, early.
    scr = stage.tile([1, 8], F32, name="scr")
    scr2 = stage.tile([1, 8], F32, name="scr2")
    nc.vector.memset(scr[:], 0.0)
    nc.scalar.activation(scr2[:], scr[:], EXP, scale=1.0)

    # ---- scoped pools, closed explicitly in LIFO order so space recycles
    es_proj = ExitStack()
    projp = es_proj.enter_context(tc.tile_pool(name="projp", bufs=1))
    xT = projp.tile([128, 8 * 1024], BF)   # [d-part, dt*1024 + l]
    wq = projp.tile([128, 8 * 1024], BF)   # [d-part, dt*1024 + e]
    wk = projp.tile([128, 8 * 1024], BF)
    es_wns = ExitStack()    # ns weight chunks: freed after ns_phase
    wnsp = es_wns.enter_context(tc.tile_pool(name="wnsp", bufs=7))
    es_pv = ExitStack()     # wv: freed after the V projections
    pvp = es_pv.enter_context(tc.tile_pool(name="pvp", bufs=1))
    wv = pvp.tile([128, 8 * 1024], BF)

    # ---- input DMAs.
    # Queue roles: scalar carries only never-blocking transfers (its later
    # stream is the exp activations); sync takes the collective-gated a2a
    # load + v-tails (idle otherwise); gpsimd takes smalls, half the wns
    # chunks, then gin + the collective trigger.  The wns chunk pool is
    # deliberately ONLY on gpsimd+sync: its WAR self-pacing freezes the
    # issuing engine until ns consumption starts, which both can afford.
    nc.gpsimd.dma_start(id8[:], aps["id8"][:])
    nc.gpsimd.dma_start(tri[:], aps["tri"][:])
    nc.gpsimd.dma_start(kpm[:], aps["kpm"][:])
    nc.gpsimd.dma_start(ktail8[:], aps["ktail8"][:])
    for dt in range(8):
        r = slice(dt * 128, dt * 128 + 128)
        nc.gpsimd.dma_start(xtails[:, bass.ts(dt, 8)], aps["xtails"][r, :])

    # priority stream on sync+scalar: xT/wv pairs (feeds the dt-major V
    # prelude), then wq, wk, wo
    chunks = []
    for dt in range(8):
        r = slice(dt * 128, dt * 128 + 128)
        chunks.append((xT[:, bass.ts(dt, 1024)], aps["xT"][r, :]))
        chunks.append((wv[:, bass.ts(dt, 1024)], aps["wvT"][r, :]))
    for name, dst in (("wqT", wq), ("wkT", wk), ("woutT", wo)):
        for dt in range(8):
            r = slice(dt * 128, dt * 128 + 128)
            chunks.append((dst[:, bass.ts(dt, 1024)], aps[name][r, :]))
    for i, (dst, src) in enumerate(chunks):
        (nc.sync if i % 2 == 0 else nc.scalar).dma_start(dst, src)

    # wns: 16 half-width chunks [128, 1536]; first 8 (dt 0-3) on gpsimd,
    # rest on scalar (its engine can afford the slot-wait stall: the first
    # exps come long after ns consumption frees the slots).  bufs=7
    # pre-stages ~2.8MB before ns consumption begins.
    wts = []
    for ci in range(16):
        wt = wnsp.tile([128, 1536], BF, name="wt")
        wts.append(wt)
        dt, half = ci // 2, ci % 2
        src = aps["wnsT"][dt * 128: dt * 128 + 128,
                          half * 1536: half * 1536 + 1536]
        (nc.gpsimd if ci < 8 else nc.scalar).dma_start(wt[:], src)

    # ---- helpers ----
    def v_copyout(lt, accs, m=128):
        # accs: {eg: psum [<=128, 512]} -> vb with kpm folded in + the
        # replicated-kpm denominator columns
        for eg in range(2):
            dstv = vb[0:m, lt * 2048 + eg * 1024: lt * 2048 + eg * 1024 + 1024].rearrange(
                "p (h x) -> p h x", h=8, x=128
            )[:, :, 64:128]
            srcv = accs[eg][0:m, :].rearrange("p (h x) -> p h x", h=8, x=64)
            nc.vector.tensor_scalar_mul(dstv, srcv, kpm[0:m, lt: lt + 1])
        dstm = vb[0:m, lt * 2048: lt * 2048 + 2048].rearrange(
            "p (h x) -> p h x", h=16, x=128
        )[:, :, 0:64]
        nc.vector.tensor_copy(
            dstm, kpm[0:m, lt: lt + 1].unsqueeze(1).to_broadcast((m, 16, 64))
        )

    def v_block(lt):
        m = 120 if lt == 7 else 128          # tail rows 1016.. come from ns
        accs = {}
        for eg in range(2):
            acc = accp.tile([128, 512], F32, name="acc")
            accs[eg] = acc
            for dt in range(8):
                nc.tensor.matmul(
                    acc[0:m, :],
                    xT[:, dt * 1024 + lt * 128: dt * 1024 + lt * 128 + m],
                    wv[:, dt * 1024 + eg * 512: dt * 1024 + eg * 512 + 512],
                    start=(dt == 0),
                    stop=(dt == 7),
                )
        v_copyout(lt, accs, m)

    def w_round(w, dst, et, ceng=None):
        # one projection round: both column halves for one et block
        for lg in range(2):
            w0 = lg * 512
            w1 = 512 if lg == 0 else 504   # tail cols come from ns
            acc = accp.tile([128, 512], F32, name="acc")
            for dt in range(8):
                nc.tensor.matmul(
                    acc[:, 0:w1],
                    w[:, dt * 1024 + et * 128: dt * 1024 + et * 128 + 128],
                    xT[:, dt * 1024 + w0: dt * 1024 + w0 + w1],
                    start=(dt == 0),
                    stop=(dt == 7),
                )
            (ceng or nc.any).tensor_copy(
                dst[:, et * 1024 + w0: et * 1024 + w0 + w1], acc[:, 0:w1])

    def attn_pair_g(r, g, scp, ptp, opp):
        # Both heads of the pair in lockstep: their score matmuls contract
        # over disjoint partition halves (r0=0 vs 64 -> row groups 0-1 vs
        # 2-3 via auto tile_position), so back-to-back issue runs them
        # concurrently on the PE array.  Scores land COMPACTED (valid
        # columns only) so each (head, jp-pair) is one exp activation.
        et = r
        hs = (2 * r, 2 * r + 1)
        nj = 4 if g == 0 else 8
        qbase = et * 1024 + g * 512
        pcs, ptw = _pcs(g)
        pts = {h: ptp.tile([128, ptw], BF, name="pt") for h in hs}
        for jp in range(0, nj, 2):
            leads = [_lead(g, jp), _lead(g, jp + 1)]
            if g == 1 and jp < 4:
                w = 1024
            else:
                w = (512 - leads[0]) + (512 - leads[1])
            scs = {h: scp.tile([128, 1024], F32, name="sc") for h in hs}
            for s in range(2):
                j = jp + s
                lead = leads[s]
                cs = 0 if s == 0 else 512 - leads[0]
                for h in hs:
                    r0 = (h % 2) * 64
                    nc.tensor.matmul(
                        scs[h][:, cs: cs + 512 - lead],
                        kT[r0:r0 + 64,
                           et * 1024 + j * 128: et * 1024 + j * 128 + 128],
                        qT[r0:r0 + 64, qbase + lead: qbase + 512],
                        start=True,
                        stop=True,
                    )
            for h in hs:
                sc, pt = scs[h], pts[h]
                if not (g == 1 and jp < 4):
                    # diagonal = first 128 valid cols of each j block
                    for s in range(2):
                        cs = 0 if s == 0 else 512 - leads[0]
                        nc.vector.tensor_tensor(
                            sc[:, cs: cs + 128],
                            sc[:, cs: cs + 128],
                            tri[:],
                            ADD,
                        )
                nc.scalar.activation(
                    pt[:, pcs[jp]: pcs[jp] + w], sc[:, 0:w], EXP, scale=0.125,
                )
        for h in hs:
            r0 = (h % 2) * 64
            pt = pts[h]
            op = opp.tile([128, 512], F32, name="op")
            for j in range(nj):
                lead = _lead(g, j)
                nc.tensor.matmul(
                    op[:, lead:512],
                    vb[:, j * 2048 + h * 128: j * 2048 + h * 128 + 128],
                    pt[:, pcs[j]: pcs[j] + 512 - lead],
                    start=(j == 0),
                    stop=(j == nj - 1),
                )
            # normalize: den = op rows 0..63 (replicated kpm columns sit
            # first in vb, and custom-DVE ops force base partition 0, so
            # the reciprocal reads PSUM row 0 directly).  Broadcast the
            # reciprocal row on gpsimd (idle engine; the collective trigger
            # ahead of it in the FIFO is fire-and-forget).
            rec1 = stage.tile([1, 512], F32, name="rec1")
            nc.vector.reciprocal_approx_fast(rec1[:], op[0:1, 0:512])
            bc = stage.tile([64, 512], F32, name="bc")
            nc.gpsimd.partition_broadcast(bc[:], rec1[:], channels=64)
            nc.vector.tensor_tensor(
                oT[r0:r0 + 64, qbase: qbase + 512],
                op[64:128, 0:512],
                bc[:],
                MULT,
            )

    def ns_mms(nsp, dts):
        # per-position projections, 4-way column-packed on the PE array
        # (tile_position col group from psum base partition).
        pp1, pp2, dsts = nsp
        for dt in dts:
            for ck in range(6):
                pp, po = dsts[ck]
                wt = wts[2 * dt + (0 if ck < 3 else 1)]
                nc.tensor.matmul(
                    pp[po:po + 8, :],
                    xtails[:, bass.ts(dt, 8)],
                    wt[:, bass.ts(ck % 3, 512)],
                    start=(dt == 0),
                    stop=(dt == 7),
                    tile_position=(0, po),
                )

    def ns_finish(nsp):
        pp1, pp2, dsts = nsp
        for ck in range(6):
            pp, po = dsts[ck]
            nc.vector.tensor_copy(nsb[:, bass.ts(ck, 512)], pp[po:po + 8, :])
        gin = dram.tile([8, 3072], BF, name="gin")
        gout = dram.tile([8, 3072], BF, name="gout")
        nc.gpsimd.dma_start(gin[:], nsb[:])
        nc.gpsimd.collective_compute(
            "AllToAll",
            mybir.AluOpType.bypass,
            replica_groups=[list(range(NCORES))],
            ins=[gin.opt()],
            outs=[gout.opt()],
        )
        nc.sync.dma_start(a2a[:], gout[:])

    def tails():
        # q/k tails: transpose a2a rows (positions) into qT/kT columns via
        # tiny identity matmuls; one batched copy per tensor.
        for ti, dst in ((0, qT), (1, kT)):
            tp = accp.tile([128, 512], F32, name="acc")
            for et in range(8):
                nc.tensor.matmul(
                    tp[:, et * 8: et * 8 + 8],
                    a2a[0:8, ti * 1024 + et * 128: ti * 1024 + et * 128 + 128],
                    id8[:],
                    start=True,
                    stop=True,
                )
            dv = dst[:, 0:8192].rearrange("p (et l) -> p et l", et=8, l=1024)
            nc.any.tensor_copy(
                dv[:, :, 1016:1024],
                tp[:, 0:64].rearrange("p (et n) -> p et n", et=8, n=8),
            )
        # v tails land in natural layout already (row n = position n); mask
        # by tail key padding, prepend the replicated kpm columns, then one
        # contiguous DMA per half.
        for vg in range(2):
            vt = stage.tile([8, 1024], BF, name="vt")
            dvt = vt[:].rearrange("p (h x) -> p h x", h=8, x=128)
            nc.vector.tensor_scalar_mul(
                dvt[:, :, 64:128],
                a2a[0:8, 2048 + vg * 512: 2048 + vg * 512 + 512].rearrange(
                    "p (h x) -> p h x", h=8, x=64
                ),
                ktail8[:],
            )
            nc.vector.tensor_copy(
                dvt[:, :, 0:64],
                ktail8[:].unsqueeze(1).to_broadcast((8, 8, 64)),
            )
            nc.sync.dma_start(
                vb[120:128, 7 * 2048 + vg * 1024: 7 * 2048 + vg * 1024 + 1024],
                vt[:],
            )

    # ---- phase schedule ----
    # V prelude, dt-major over lt 0-2: paced to the interleaved xT/wv
    # chunk arrivals so the PE has work during the input-DMA window.
    NPRE = 3
    with tc.tile_pool(name="vvp", bufs=1, space="PSUM") as vvp:
        vacc = {(lt, eg): vvp.tile([128, 512], F32, name=f"va{lt}{eg}")
                for lt in range(NPRE) for eg in range(2)}
        for dt in range(8):
            for (lt, eg), acc in vacc.items():
                nc.tensor.matmul(
                    acc[:],
                    xT[:, dt * 1024 + lt * 128: dt * 1024 + lt * 128 + 128],
                    wv[:, dt * 1024 + eg * 512: dt * 1024 + eg * 512 + 512],
                    start=(dt == 0),
                    stop=(dt == 7),
                )
        for lt in range(NPRE):
            v_copyout(lt, {eg: vacc[(lt, eg)] for eg in range(2)})
    for lt in range(NPRE, 8):
        v_block(lt)
    es_pv.close()

    # q projections for all et (collective-independent PE work); ns
    # matmuls sit between rounds so consumption paces the wns chunk DMAs.
    with tc.tile_pool(name="nsps", bufs=1, space="PSUM") as nsp_pool:
        pp1 = nsp_pool.tile([128, 512], F32, name="pp1")
        pp2 = nsp_pool.tile([128, 512], F32, name="pp2")
        nsp = (pp1, pp2, [(pp1, 0), (pp1, 32), (pp1, 64), (pp1, 96),
                          (pp2, 0), (pp2, 32)])
        w_round(wq, qT, 0)
        ns_mms(nsp, range(0, 4))
        w_round(wq, qT, 1)
        ns_mms(nsp, range(4, 8))
        ns_finish(nsp)
        for et in range(2, 8):
            w_round(wq, qT, et)
    es_wns.close()

    with tc.tile_pool(name="ptp", bufs=4) as ptp, \
         tc.tile_pool(name="scp", bufs=2, space="PSUM") as scp, \
         tc.tile_pool(name="opp", bufs=2, space="PSUM") as opp:

        def outproj_lt(lt):
            # full 8-et contraction for one q-position block; lt 0-3 touch
            # only the g0 half of oT, lt 4-7 only after all g1 rounds.
            for eg in range(2):
                acc = accp.tile([128, 512], F32, name="acc")
                for et in range(8):
                    nc.tensor.matmul(
                        acc[:],
                        oT[:, et * 1024 + lt * 128: et * 1024 + lt * 128 + 128],
                        wo[:, et * 1024 + eg * 512: et * 1024 + eg * 512 + 512],
                        start=(et == 0),
                        stop=(et == 7),
                    )
                ys = stage.tile([128, 512], BF, name="ys")
                nc.vector.tensor_copy(ys[:], acc[:])
                eng = nc.gpsimd if eg == 0 else nc.scalar
                eng.dma_start(
                    aps["y"][lt * 128: lt * 128 + 128, eg * 512: eg * 512 + 512],
                    ys[:],
                )

        # k-round r feeds attn-g0 round r.  outproj lt 0-1 (pure g0
        # consumers) buffer the PE past the AllToAll completion variance
        # before tails() is consumed; lt 2-3 fill ACT-bound g1 rounds.
        w_round(wk, kT, 0, nc.vector)
        for r in range(8):
            if r < 7:
                w_round(wk, kT, r + 1, nc.vector)
            attn_pair_g(r, 0, scp, ptp, opp)
        outproj_lt(0)
        outproj_lt(1)
        tails()
        for r in range(8):
            attn_pair_g(r, 1, scp, ptp, opp)
            if r in (4, 5):
                outproj_lt(r - 2)
        for lt in range(4, 8):
            outproj_lt(lt)

    es_proj.close()

    if DEBUG:
        for nm, t in (("dqT", qT), ("dkT", kT), ("dvb", vb), ("doT", oT),
                      ("da2a", a2a), ("dnsb", nsb)):
            nc.gpsimd.dma_start(aps[nm][:, :], t[:])


def _build():
    if "nc" in _CACHE:
        return _CACHE["nc"]
    nc = bacc.Bacc("TRN2", target_bir_lowering=False, debug=False, num_devices=NCORES)
    aps = {}
    for name, shape, dt in [
        ("xT", [1024, 1024], BF),
        ("wqT", [1024, 1024], BF),
        ("wkT", [1024, 1024], BF),
        ("wvT", [1024, 1024], BF),
        ("woutT", [1024, 1024], BF),
        ("wnsT", [1024, 3072], BF),
        ("xtails", [1024, 8], BF),
        ("id8", [8, 8], BF),
        ("tri", [128, 128], F32),
        ("kpm", [128, 8], F32),
        ("ktail8", [8, 1], F32),
    ]:
        aps[name] = nc.dram_tensor(name, shape, dt, kind="ExternalInput").ap()
    aps["y"] = nc.dram_tensor("y", [1024, 1024], BF, kind="ExternalOutput").ap()
    if DEBUG:
        for nm, shape in (("dqT", [128, 8192]), ("dkT", [128, 8192]),
                          ("dvb", [128, 16384]), ("doT", [128, 8192]),
                          ("da2a", [8, 3072]), ("dnsb", [8, 3072])):
            aps[nm] = nc.dram_tensor(nm, shape, BF, kind="ExternalOutput").ap()

    with tile.TileContext(nc) as tc:
        _attention_kernel(tc, aps)
    nc.compile()
    _CACHE["nc"] = nc
    return nc


def kernel(x, key_padding_mask, Wq_s, Wk_s, Wv_s, Wq_ns, Wk_ns, Wv_ns, W_out, **kw):
    x = np.asarray(x, np.float32)
    mask = np.asarray(key_padding_mask)
    bf = ml_dtypes.bfloat16

    wqT = np.ascontiguousarray(np.asarray(Wq_s, np.float32).T.astype(bf))
    wkT = np.ascontiguousarray(np.asarray(Wk_s, np.float32).T.astype(bf))
    wvT = np.ascontiguousarray(np.asarray(Wv_s, np.float32).T.astype(bf))
    woT = np.ascontiguousarray(np.asarray(W_out, np.float32).T.astype(bf))
    tri = np.where(
        np.arange(128)[:, None] <= np.arange(128)[None, :], 0.0, NEG
    ).astype(np.float32)
    id8 = np.eye(8, dtype=bf)

    Wq_ns = np.asarray(Wq_ns, np.float32)
    Wk_ns = np.asarray(Wk_ns, np.float32)
    Wv_ns = np.asarray(Wv_ns, np.float32)

    in_maps = []
    for c in range(NCORES):
        xT = np.ascontiguousarray(x[c].T.astype(bf))
        xtails = np.ascontiguousarray(x[:, LS + c, :].T.astype(bf))  # [1024 d, 8 bb]
        wnsT = np.ascontiguousarray(
            np.concatenate([Wq_ns[c].T, Wk_ns[c].T, Wv_ns[c].T], axis=1).astype(bf)
        )
        mk = mask[c].astype(np.float32)                 # [1024] 0/1
        kpm = np.ascontiguousarray(mk.reshape(8, 128).T)   # [128 key, lt]
        ktail8 = np.ascontiguousarray(mk[LS:].reshape(8, 1))
        in_maps.append(
            {
                "xT": xT,
                "wqT": wqT,
                "wkT": wkT,
                "wvT": wvT,
                "woutT": woT,
                "wnsT": wnsT,
                "xtails": xtails,
                "id8": id8,
                "tri": tri,
                "kpm": kpm,
                "ktail8": ktail8,
            }
        )

    nc = _build()
    res = run_bass_kernel_spmd(nc, in_maps, list(range(NCORES)), trace=TRACE)
    _CACHE["exec_time_ns"] = res.exec_time_ns
    _CACHE["res"] = res
    out = np.stack([np.asarray(res.results[c]["y"]) for c in range(NCORES)], axis=0)
    return out.astype(np.float32)
